# revision 29
# baseline (speedup 1.0000x reference)
"""GAT (2-layer graph attention + pair scoring) on 8 TRN2 NeuronCores.

Sharding: destination-node rows (4096/8=512 per core). Per layer, each core
computes Wh for its rows (bf16 matmuls), scales by q=exp(0.2*e_dst),
AllGathers the scaled [N, nhid(+aux)] matrix, then computes masked attention
for its 512 rows against all 4096 sources. Pair scoring shards the 65536
pairs over cores and gathers embeddings with chunked SWDGE dma_gather.

Key algebra: exp(leaky_relu(s)) with s = e_src_i + e_dst_j factors as
  p_i * q_j * max(a_i*b_j, 1),  a=exp(.8 e_src), b=exp(.8 e_dst),
  p=exp(.2 e_src), q=exp(.2 e_dst)
and p_i cancels between softmax numerator and denominator. So the N^2 stage
needs NO transcendentals: one dual-op tensor_scalar (mult+max, DVE 4x mode)
and one tensor_tensor (mask multiply) per tile. The softmax denominator
rides along as an extra q-column in the matmul's stationary operand.

Layer-1 AG payload: per-head blocks [Whq_h (64) | q_h] then 8 b columns.
Each head's attention matmul (lhsT = [Whq_h | q_h]) leaves numerators on PSUM
partitions 0..63 and the softmax denominator on partition 64. The mask
multiplies are split DVE/GPSIMD (heads 3,7 + half of 5 on GPSIMD); PSUM
evacuation rides the otherwise-idle Activation engine.

Final embeddings go out as ONE fused bf16 AllGather [N, 128] = [h | h@Ws^T];
pair rows come back via dma_gather in 512-index chunks (HW SWDGE ring limit)
alternating two SWDGE queues, ~7x faster than per-128-row indirect DMAs.
"""

import sys

if "/opt/trn_rl_repo" not in sys.path:
    sys.path.insert(0, "/opt/trn_rl_repo")

import numpy as np
import ml_dtypes

import concourse.bacc as bacc
import concourse.tile as tile
import concourse.mybir as mybir

BF16 = mybir.dt.bfloat16
F32 = mybir.dt.float32
I16 = mybir.dt.int16
AF = mybir.ActivationFunctionType
OP = mybir.AluOpType
AX = mybir.AxisListType

N, NFEAT, NHID, NHEADS = 4096, 512, 64, 8
P = 65536
NCORES = 8
R = N // NCORES          # rows (destination nodes) per core = 512
JT = N // 128            # source j-tiles = 32
PC = P // NCORES         # pairs per core = 8192
CH = PC // 128           # pair chunks = 64
HB = NHID + 1            # per-head AG1 block [Whq (64) | q] = 65
AG1C = NHEADS * HB + NHEADS   # 520 + 8 trailing b columns = 528
AG2C = NHID + 2          # [Whq2 (64) | q2 | b2] = 66

# heads whose mask-multiply runs on GPSIMD instead of DVE (load balance)
GP_HEADS = (3, 7)
GP_JT2 = 3               # in layer 2, every GP_JT2-th j-tile's mask-mul on gpsimd


GCHK = 512               # indices per dma_gather call (HW SWDGE ring limit)
GNC = PC // GCHK         # gather chunks per table = 16


def _build_nc(stage=99, iters=1):
    nc = bacc.Bacc("TRN2", target_bir_lowering=False, debug=False,
                   num_devices=NCORES, num_swdge_queues=2)

    def inp(name, shape, dt):
        return nc.dram_tensor(name, shape, dt, kind="ExternalInput").ap()

    xT = inp("xT", [NFEAT, R], BF16)           # x[rows].T  (feature-major)
    maskT = inp("maskT", [N, R], BF16)         # adj[rows].T (0/1)
    Wcat = inp("Wcat", [NFEAT, NHEADS * NHID], BF16)
    Asrc = inp("Asrc", [NHEADS * NHID, NHEADS], BF16)  # block-diag a_src
    Adst = inp("Adst", [NHEADS * NHID, NHEADS], BF16)  # block-diag a_dst
    Wout = inp("Wout", [NHEADS * NHID, NHID], BF16)
    aout2 = inp("aout2", [NHID, 2], BF16)      # col0 = a_out[:64], col1 = a_out[64:]
    WsT = inp("WsT", [NHID, NHID], F32)        # W_score.T
    ident = inp("ident", [128, 128], F32)
    selbc = inp("selbc", [NHEADS, R], F32)     # selbc[h, m] = (m//64 == h)
    idx1 = inp("idx1", [128, PC // 16], I16)   # dma_gather 16-partition wrap
    idx2 = inp("idx2", [128, PC // 16], I16)

    scores = nc.dram_tensor("scores", [128, CH], F32, kind="ExternalOutput").ap()

    rg = [list(range(NCORES))]

    with tile.TileContext(nc) as tc:
        with tc.tile_pool(name="sb", bufs=1) as sb, \
             tc.tile_pool(name="sbw", bufs=12) as sbw, \
             tc.tile_pool(name="ps", bufs=8, space="PSUM") as ps, \
             tc.tile_pool(name="dram", bufs=1, space="DRAM") as dram:

            for _it in range(iters):
                def pst(name):
                    return ps.tile([128, R], F32, tag="ps", name=name)

                # ---------- persistent loads ----------
                Wout_sb = sb.tile([128, 4, NHID], BF16, tag="Wout")
                nc.sync.dma_start(Wout_sb[:], Wout.rearrange("(k p) c -> p k c", p=128))
                aout2_sb = sb.tile([NHID, 2], BF16, tag="aout2")
                nc.sync.dma_start(aout2_sb[:], aout2[:])
                WsT_sb = sb.tile([NHID, NHID], F32, tag="WsT")
                nc.sync.dma_start(WsT_sb[:], WsT[:])
                ident_sb = sb.tile([128, 128], F32, tag="ident")
                nc.sync.dma_start(ident_sb[:], ident[:])
                selbc_sb = sb.tile([NHEADS, R], F32, tag="selbc")
                nc.sync.dma_start(selbc_sb[:], selbc[:])
                ones_sb = sb.tile([65, 128], F32, tag="ones")
                nc.vector.memset(ones_sb[:], 1.0)


                abc_sb = sb.tile([128, NHEADS, R], BF16, tag="abc")
                ag1_sb = [sb.tile([128, JT // 4, AG1C], BF16,
                                  tag=f"ag1sb{_q}", name=f"ag1sb{_q}")
                          for _q in range(4)]
                bf_sb = sb.tile([128, JT, NHEADS], F32, tag="bf")
                hcatT_sb = [sb.tile([128, R], BF16, tag=f"hcatT{_g}",
                                    name=f"hcatT{_g}") for _g in range(4)]
                num_sb = sb.tile([128, 4, R], BF16, tag="num")

                ag1_in = dram.tile([R, AG1C], BF16, tag="ag1in")
                ag1_out = dram.tile([N, AG1C], BF16, tag="ag1out", addr_space="Shared")

                # ---------- Phase A: local Wh / e / exps / AG1 payload ----------
                with tc.tile_pool(name="sbA", bufs=1) as sbA:
                    xT_sb = sbA.tile([128, 4, R], BF16, tag="xT")
                    nc.sync.dma_start(
                        xT_sb[:, 0:2, :],
                        xT[0:NFEAT // 2].rearrange("(k p) c -> p k c", p=128))
                    nc.sync.dma_start(
                        xT_sb[:, 2:4, :],
                        xT[NFEAT // 2:NFEAT].rearrange("(k p) c -> p k c", p=128))
                    Wcat_sb = sbA.tile([128, 4, NHEADS * NHID], BF16, tag="Wcat")
                    nc.sync.dma_start(
                        Wcat_sb[:, 0:2, :],
                        Wcat[0:NFEAT // 2].rearrange("(k p) c -> p k c", p=128))
                    nc.sync.dma_start(
                        Wcat_sb[:, 2:4, :],
                        Wcat[NFEAT // 2:NFEAT].rearrange("(k p) c -> p k c", p=128))
                    Asrc_sb = sbA.tile([128, 4, NHEADS], BF16, tag="Asrc")
                    nc.sync.dma_start(Asrc_sb[:],
                                      Asrc.rearrange("(k p) c -> p k c", p=128))
                    Adst_sb = sbA.tile([128, 4, NHEADS], BF16, tag="Adst")
                    nc.sync.dma_start(Adst_sb[:],
                                      Adst.rearrange("(k p) c -> p k c", p=128))

                    # bulk loads not needed until phase B, issued after
                    # phase A's inputs so they don't gate the first matmuls
                    maskT_sb = sb.tile([128, JT, R], BF16, tag="maskT")
                    nc.sync.dma_start(maskT_sb[:],
                                      maskT.rearrange("(j p) c -> p j c", p=128))
                    idx1_sb = sb.tile([128, PC // 16], I16, tag="idx1")
                    nc.sync.dma_start(idx1_sb[:], idx1[:])
                    idx2_sb = sb.tile([128, PC // 16], I16, tag="idx2")
                    nc.sync.dma_start(idx2_sb[:], idx2[:])

                    # Wh row-major [512_i, 512_hd] and WhT [512_hd, 512_i]
                    wh_sb = sbA.tile([128, 4, NHEADS * NHID], BF16, tag="wh")
                    whT_sb = sbA.tile([128, 4, R], BF16, tag="whT")
                    for m in range(4):
                        wh_ps = pst(f"whps{m}")
                        for k in range(4):
                            nc.tensor.matmul(wh_ps[:],
                                             xT_sb[:, k, m * 128:(m + 1) * 128],
                                             Wcat_sb[:, k, :],
                                             start=(k == 0), stop=(k == 3))
                        nc.scalar.copy(wh_sb[:, m, :], wh_ps[:])
                        whT_ps = pst(f"whTps{m}")
                        for k in range(4):
                            nc.tensor.matmul(whT_ps[:],
                                             Wcat_sb[:, k, m * 128:(m + 1) * 128],
                                             xT_sb[:, k, :],
                                             start=(k == 0), stop=(k == 3))
                        nc.scalar.copy(whT_sb[:, m, :], whT_ps[:])

                    # abc[h] = exp(0.8*e_src_h) broadcast to all partitions:
                    # stride-0 lhsT makes every output partition identical
                    for h in range(NHEADS):
                        ebc_ps = pst(f"ebc{h}")
                        for k in range(4):
                            nc.tensor.matmul(
                                ebc_ps[:],
                                Asrc_sb[:, k, h:h + 1].to_broadcast([128, 128]),
                                whT_sb[:, k, :],
                                start=(k == 0), stop=(k == 3))
                        nc.scalar.activation(abc_sb[:, h, :], ebc_ps[:], AF.Exp,
                                             scale=0.8)

                    # e_dstT [512_i, 8] -> qT (f32, scalar use) and bT (bf16)
                    qT_sb = sbA.tile([128, 4, NHEADS], F32, tag="qT")
                    bT_sb = sbA.tile([128, 4, NHEADS], BF16, tag="bT")
                    for m in range(4):
                        ed_ps = pst(f"ed{m}")
                        for k in range(4):
                            nc.tensor.matmul(ed_ps[:, 0:NHEADS],
                                             whT_sb[:, k, m * 128:(m + 1) * 128],
                                             Adst_sb[:, k, :],
                                             start=(k == 0), stop=(k == 3))
                        nc.scalar.activation(qT_sb[:, m, :], ed_ps[:, 0:NHEADS],
                                             AF.Exp, scale=0.2)
                        nc.scalar.activation(bT_sb[:, m, :], ed_ps[:, 0:NHEADS],
                                             AF.Exp, scale=0.8)

                    # AG1 payload: per-head [Whq_h | q_h] blocks + 8 b cols;
                    # per-m tiles so each chunk's DMA streams out while the
                    # next chunk is still being assembled
                    for m in range(4):
                        pay1m = sbA.tile([128, AG1C], BF16, tag=f"pay1{m}",
                                         name=f"pay1{m}")
                        for h in range(NHEADS):
                            nc.vector.tensor_scalar(
                                pay1m[:, h * HB:h * HB + NHID],
                                wh_sb[:, m, h * NHID:(h + 1) * NHID],
                                qT_sb[:, m, h:h + 1], None, OP.mult)
                            nc.vector.tensor_copy(
                                pay1m[:, h * HB + NHID:h * HB + NHID + 1],
                                qT_sb[:, m, h:h + 1])
                        nc.vector.tensor_copy(pay1m[:, NHEADS * HB:AG1C],
                                              bT_sb[:, m, :])
                        nc.sync.dma_start(ag1_in[m * 128:(m + 1) * 128, :],
                                          pay1m[:])

                nc.gpsimd.collective_compute(
                    "AllGather", OP.bypass, replica_groups=rg,
                    ins=[ag1_in.opt()], outs=[ag1_out.opt()])
                for _q in range(4):
                    nc.sync.dma_start(
                        ag1_sb[_q][:],
                        ag1_out[_q * (N // 4):(_q + 1) * (N // 4)].rearrange(
                            "(j p) c -> p j c", p=128))
                    nc.scalar.copy(
                        bf_sb[:, _q * (JT // 4):(_q + 1) * (JT // 4), :],
                        ag1_sb[_q][:, :, NHEADS * HB:AG1C])

                if stage == 1:
                    dmy = sb.tile([128, CH], F32, tag="dmy", name="dmy1")
                    nc.vector.memset(dmy[:], 0.0)
                    nc.sync.dma_start(scores[:], dmy[:])
                    return nc

                # ---------- Phase B: layer-1 attention ----------
                # lhsT = [Whq_h | q_h]: numerators on psum p0..63, denom on p64
                hp_ps = [ps.tile([65, R], F32, tag="ps", name=f"hp{h}")
                         for h in range(NHEADS)]
                for j in range(JT):
                    for h in range(NHEADS):
                        t = sbw.tile([128, R], BF16, tag="t")
                        nc.vector.tensor_scalar(
                            t[:], abc_sb[:, h, :], bf_sb[:, j, h:h + 1],
                            1.0, OP.mult, OP.max)
                        t2 = sbw.tile([128, R], BF16, tag="t2")
                        on_gp = h in GP_HEADS or (h == 5 and j % 2 == 0)
                        eng = nc.gpsimd if on_gp else nc.vector
                        eng.tensor_tensor(t2[:], t[:], maskT_sb[:, j, :], OP.mult)
                        nc.tensor.matmul(
                            hp_ps[h][:],
                            ag1_sb[j // (JT // 4)][:, j % (JT // 4),
                                                   h * HB:h * HB + NHID + 1],
                            t2[:], start=(j == 0), stop=(j == JT - 1))


                if stage == 2:
                    dmy = sb.tile([128, CH], F32, tag="dmy", name="dmy2")
                    nc.vector.memset(dmy[:], 0.0)
                    nc.sync.dma_start(scores[:], dmy[:])
                    return nc
                # normalize + elu -> hcatT [512_hd, 512_i] bf16
                with tc.tile_pool(name="sbE", bufs=1) as sbE:
                    den_st = sbE.tile([65, NHEADS, R], F32, tag="den_st")
                    for h in range(NHEADS):
                        nc.scalar.copy(den_st[64:65, h, :],
                                       hp_ps[h][64:65, :])
                        if h % 2 == 0:
                            nc.scalar.copy(num_sb[0:64, h // 2, :],
                                           hp_ps[h][0:64, :])
                        else:
                            nc.vector.tensor_copy(num_sb[64:128, h // 2, :],
                                                  hp_ps[h][0:64, :])
                    rin_sb = sbE.tile([NHEADS, R], F32, tag="rin")
                    nc.sync.dma_start(rin_sb[:], den_st[64:65, :, :])
                    rcp_sb = sbE.tile([NHEADS, R], F32, tag="rcp")
                    nc.vector.reciprocal(rcp_sb[:], rin_sb[:])

                    # per-chunk tiles: norm+elu stream into phase C's
                    # accumulating matmuls chunk-by-chunk
                    with tc.tile_pool(name="sbG", bufs=2) as sbG:
                        for g in range(4):
                            rbc_ps = pst(f"rbc{g}")
                            nc.tensor.matmul(rbc_ps[:],
                                             selbc_sb[:, g * 128:(g + 1) * 128],
                                             rcp_sb[:], start=True, stop=True)
                            xbg = sbG.tile([128, R], BF16, tag="xbg",
                                           name=f"xbg{g}")
                            nc.vector.tensor_tensor(xbg[:], num_sb[:, g, :],
                                                    rbc_ps[:], OP.mult)
                            # elu(x) = relu(x) + exp(min(x,0)) - 1
                            tmin = sbG.tile([128, R], BF16, tag="tming",
                                            name=f"tmin{g}")
                            nc.vector.tensor_scalar(tmin[:], xbg[:], 0.0, None,
                                                    OP.min)
                            texp = sbG.tile([128, R], BF16, tag="texpg",
                                            name=f"texp{g}")
                            nc.scalar.activation(texp[:], tmin[:], AF.Exp)
                            trel = sbG.tile([128, R], BF16, tag="trelg",
                                            name=f"trel{g}")
                            nc.vector.tensor_scalar(trel[:], xbg[:], 0.0, 1.0,
                                                    OP.max, OP.subtract)
                            nc.vector.tensor_tensor(hcatT_sb[g][:], texp[:],
                                                    trel[:], OP.add)

                # ---------- Phase C: layer-2 ingredients + AG2 ----------
                ag2_in = dram.tile([R, AG2C], BF16, tag="ag2in")
                ag2_out = dram.tile([N, AG2C], BF16, tag="ag2out", addr_space="Shared")
                wh2T_sb = sb.tile([NHID, R], BF16, tag="wh2Tsb")
                a2bc_sb = sb.tile([128, R], BF16, tag="a2bcsb")
                with tc.tile_pool(name="sbC", bufs=1) as sbC:
                    wh2T_ps = ps.tile([NHID, R], F32, tag="ps", name="wh2T")
                    for k in range(4):
                        nc.tensor.matmul(wh2T_ps[:], Wout_sb[:, k, :],
                                         hcatT_sb[k][:],
                                         start=(k == 0), stop=(k == 3))
                    nc.scalar.copy(wh2T_sb[:], wh2T_ps[:])

                    a2e_ps = pst("a2e")
                    nc.tensor.matmul(a2e_ps[:],
                                     aout2_sb[:, 0:1].to_broadcast([NHID, 128]),
                                     wh2T_sb[:], start=True, stop=True)
                    nc.scalar.activation(a2bc_sb[:], a2e_ps[:], AF.Exp, scale=0.8)

                    # AG2 payload block: [Whq2 (64) | q2 | b2]
                    pay2_sb = sbC.tile([128, 4, AG2C], BF16, tag="pay2")
                    for m in range(4):
                        wh2_ps = ps.tile([128, NHID], F32, tag="ps",
                                         name=f"wh2_{m}")
                        for k in range(4):
                            nc.tensor.matmul(wh2_ps[:],
                                             hcatT_sb[k][:, m * 128:(m + 1) * 128],
                                             Wout_sb[:, k, :],
                                             start=(k == 0), stop=(k == 3))
                        ed2_ps = ps.tile([128, 1], F32, tag="ps", name=f"ed2_{m}")
                        nc.tensor.matmul(ed2_ps[:],
                                         wh2T_sb[:, m * 128:(m + 1) * 128],
                                         aout2_sb[:, 1:2], start=True, stop=True)
                        q2f = sbC.tile([128, 4], F32, tag="q2f", bufs=4,
                                       name=f"q2f_{m}")
                        nc.scalar.activation(q2f[:, 0:1], ed2_ps[:], AF.Exp,
                                             scale=0.2)
                        nc.vector.tensor_copy(pay2_sb[:, m, NHID:NHID + 1],
                                              q2f[:, 0:1])
                        nc.scalar.activation(pay2_sb[:, m, NHID + 1:NHID + 2],
                                             ed2_ps[:], AF.Exp, scale=0.8)
                        nc.vector.tensor_scalar(pay2_sb[:, m, 0:NHID],
                                                wh2_ps[:], q2f[:, 0:1],
                                                None, OP.mult)
                    nc.sync.dma_start(
                        ag2_in[:].rearrange("(m p) c -> p m c", p=128), pay2_sb[:])

                nc.gpsimd.collective_compute(
                    "AllGather", OP.bypass, replica_groups=rg,
                    ins=[ag2_in.opt()], outs=[ag2_out.opt()])
                ag2_sb = sb.tile([128, JT, AG2C], BF16, tag="ag2sb")
                nc.sync.dma_start(ag2_sb[:],
                                  ag2_out[:].rearrange("(j p) c -> p j c", p=128))
                b2f_sb = sb.tile([128, JT, 1], F32, tag="b2f")
                nc.scalar.copy(b2f_sb[:], ag2_sb[:, :, NHID + 1:NHID + 2])

                if stage == 3:
                    dmy = sb.tile([128, CH], F32, tag="dmy", name="dmy3")
                    nc.vector.memset(dmy[:], 0.0)
                    nc.sync.dma_start(scores[:], dmy[:])
                    return nc

                # ---------- Phase D: layer-2 attention ----------
                hp2_ps = ps.tile([65, R], F32, tag="ps", name="hp2")
                for j in range(JT):
                    t = sbw.tile([128, R], BF16, tag="t")
                    nc.vector.tensor_scalar(t[:], a2bc_sb[:], b2f_sb[:, j, 0:1],
                                            1.0, OP.mult, OP.max)
                    t2 = sbw.tile([128, R], BF16, tag="t2")
                    eng = nc.gpsimd if (j % GP_JT2 == GP_JT2 - 1) else nc.vector
                    eng.tensor_tensor(t2[:], t[:], maskT_sb[:, j, :], OP.mult)
                    nc.tensor.matmul(hp2_ps[:], ag2_sb[:, j, 0:NHID + 1], t2[:],
                                     start=(j == 0), stop=(j == JT - 1))


                if stage == 4:
                    dmy = sb.tile([128, CH], F32, tag="dmy", name="dmy4")
                    nc.vector.memset(dmy[:], 0.0)
                    nc.sync.dma_start(scores[:], dmy[:])
                    return nc
                hfT_sb = sb.tile([NHID, R], F32, tag="hfT")
                with tc.tile_pool(name="sbD", bufs=1) as sbD:
                    rcp2_sb = sbD.tile([65, R], F32, tag="rcp2")
                    nc.vector.reciprocal(rcp2_sb[64:65, :], hp2_ps[64:65, :])
                    rbc2_ps = ps.tile([NHID, R], F32, tag="ps", name="rbc2")
                    nc.tensor.matmul(rbc2_ps[:], ones_sb[64:65, 0:NHID],
                                     rcp2_sb[64:65, :], start=True, stop=True)
                    num2_sb = sbD.tile([NHID, R], F32, tag="num2")
                    nc.scalar.copy(num2_sb[:], hp2_ps[0:NHID, :])
                    xn2_sb = sbD.tile([NHID, R], F32, tag="xn2")
                    nc.vector.tensor_tensor(xn2_sb[:], num2_sb[:],
                                            rbc2_ps[:], OP.mult)
                    # elu in f32
                    tmin2 = sbD.tile([NHID, R], F32, tag="tmin2")
                    nc.vector.tensor_scalar(tmin2[:], xn2_sb[:], 0.0, None, OP.min)
                    texp2 = sbD.tile([NHID, R], F32, tag="texp2")
                    nc.scalar.activation(texp2[:], tmin2[:], AF.Exp)
                    trel2 = sbD.tile([NHID, R], F32, tag="trel2")
                    nc.vector.tensor_scalar(trel2[:], xn2_sb[:], 0.0, 1.0, OP.max,
                                            OP.subtract)
                    nc.vector.tensor_tensor(hfT_sb[:], texp2[:], trel2[:], OP.add)

                # ---------- Phase E: H2 = h @ Ws^T, transpose h, AG3 ----------
                # single fused collective: cols [0:64] = h rows, [64:128] = h@Ws^T
                ag3_in = dram.tile([R, 2 * NHID], BF16, tag="ag3in")
                ag3_out = dram.tile([N, 2 * NHID], BF16, tag="ag3out",
                                    addr_space="Shared")
                with tc.tile_pool(name="sbF", bufs=1) as sbF:
                    ag3_sb = sbF.tile([128, 4, 2 * NHID], BF16, tag="ag3")
                    for m in range(4):
                        h2_ps = ps.tile([128, NHID], F32, tag="ps", name=f"h2_{m}")
                        nc.tensor.matmul(h2_ps[:], hfT_sb[:, m * 128:(m + 1) * 128],
                                         WsT_sb[:], start=True, stop=True)
                        nc.scalar.copy(ag3_sb[:, m, NHID:2 * NHID], h2_ps[:])
                        hf_ps = ps.tile([128, NHID], F32, tag="ps", name=f"hf_{m}")
                        nc.tensor.transpose(hf_ps[:],
                                            hfT_sb[:, m * 128:(m + 1) * 128],
                                            ident_sb[0:NHID, 0:NHID])
                        nc.scalar.copy(ag3_sb[:, m, 0:NHID], hf_ps[:])

                    nc.sync.dma_start(
                        ag3_in[:].rearrange("(m p) c -> p m c", p=128), ag3_sb[:])
                    nc.gpsimd.collective_compute(
                        "AllGather", OP.bypass, replica_groups=rg,
                        ins=[ag3_in.opt()], outs=[ag3_out.opt()])


                    if stage == 5:
                        dmy = sbF.tile([128, CH], F32, tag="dmy", name="dmy5")
                        nc.vector.memset(dmy[:], 0.0)
                        nc.sync.dma_start(scores[:], dmy[:])
                        return nc
                    # ---------- Phase F: pair gather + bilinear score ----------
                    # dma_gather: 512-idx chunks (SWDGE ring limit), the two
                    # tables on separate SWDGE queues for parallel desc-gen
                    # per-quarter tiles: Tile tracks them independently, so
                    # each quarter's score math overlaps later quarters'
                    # gather descriptor-gen on the Pool engine
                    g1_sb = [sbF.tile([128, CH // 4, 2 * NHID], BF16,
                                      tag=f"g1_{_q}", name=f"g1_{_q}")
                             for _q in range(4)]
                    g2_sb = [sbF.tile([128, CH // 4, 2 * NHID], BF16,
                                      tag=f"g2_{_q}", name=f"g2_{_q}")
                             for _q in range(4)]
                    for c in range(GNC):
                        q, cq = c // 4, c % 4
                        cc = slice(cq * (GCHK // 128), (cq + 1) * (GCHK // 128))
                        ci = slice(c * (GCHK // 16), (c + 1) * (GCHK // 16))
                        nc.gpsimd.dma_gather(
                            g1_sb[q][:, cc, :], ag3_out[:], idx1_sb[:, ci],
                            GCHK, GCHK, 2 * NHID, queue_num=0)
                        nc.gpsimd.dma_gather(
                            g2_sb[q][:, cc, :], ag3_out[:], idx2_sb[:, ci],
                            GCHK, GCHK, 2 * NHID, queue_num=1)

                    if stage == 6:
                        dmy = sbF.tile([128, CH], F32, tag="dmy", name="dmy6")
                        nc.vector.memset(dmy[:], 0.0)
                        nc.sync.dma_start(scores[:], dmy[:])
                        return nc
                    sc_sb = sbF.tile([128, CH], F32, tag="sc")
                    for q in range(4):
                        prod_sb = sbF.tile([128, CH // 4, NHID], BF16,
                                           tag=f"prod{q}", name=f"prod{q}")
                        nc.vector.tensor_tensor(prod_sb[:],
                                                g1_sb[q][:, :, 0:NHID],
                                                g2_sb[q][:, :, NHID:2 * NHID],
                                                OP.mult)
                        nc.vector.tensor_reduce(
                            sc_sb[:, q * (CH // 4):(q + 1) * (CH // 4)],
                            prod_sb[:], AX.X, OP.add)
                    nc.sync.dma_start(scores[:], sc_sb[:])

    return nc


def _make_in_maps(x, adj, W_heads, a_heads, W_out, a_out, W_score,
                  pair1_idx, pair2_idx):
    bf = ml_dtypes.bfloat16
    x = np.asarray(x, dtype=np.float32)
    adj = np.asarray(adj, dtype=np.float32)
    W_heads = np.asarray(W_heads, dtype=np.float32)
    a_heads = np.asarray(a_heads, dtype=np.float32)
    W_out = np.asarray(W_out, dtype=np.float32)
    a_out = np.asarray(a_out, dtype=np.float32)
    W_score = np.asarray(W_score, dtype=np.float32)
    pair1_idx = np.asarray(pair1_idx, dtype=np.int32)
    pair2_idx = np.asarray(pair2_idx, dtype=np.int32)

    Wcat = np.concatenate([W_heads[h] for h in range(NHEADS)], axis=1)
    Wcat = np.ascontiguousarray(Wcat, dtype=np.float32)
    Asrc = np.zeros((NHEADS * NHID, NHEADS), dtype=np.float32)
    Adst = np.zeros((NHEADS * NHID, NHEADS), dtype=np.float32)
    for h in range(NHEADS):
        Asrc[h * NHID:(h + 1) * NHID, h] = a_heads[h, :NHID]
        Adst[h * NHID:(h + 1) * NHID, h] = a_heads[h, NHID:]
    Wout_bf = W_out.astype(bf)
    aout2 = np.stack([a_out[:NHID], a_out[NHID:]], axis=1).astype(bf)
    WsT = np.ascontiguousarray(W_score.T, dtype=np.float32)
    ident = np.eye(128, dtype=np.float32)
    selbc = np.zeros((NHEADS, R), dtype=np.float32)
    for h in range(NHEADS):
        selbc[h, h * NHID:(h + 1) * NHID] = 1.0

    # dma_gather slot i lands at dst[i % 128, (i // 512) * 4 + (i % 512) // 128]
    # permute so dst[p, ch] = pair p * CH + ch (same layout the unpack expects),
    # then wrap in the 16-partition index layout replicated over 8 groups
    i_arr = np.arange(PC)
    gperm = (i_arr % 128) * CH + (i_arr // GCHK) * (GCHK // 128) \
        + (i_arr % GCHK) // 128

    def gidx(ids):
        g = ids[gperm].astype(np.int16)
        return np.ascontiguousarray(
            np.tile(g.reshape(PC // 16, 16).T, (8, 1)))

    in_maps = []
    for c in range(NCORES):
        rows = slice(c * R, (c + 1) * R)
        in_maps.append(dict(
            xT=np.ascontiguousarray(x[rows].T).astype(bf),
            maskT=np.ascontiguousarray(adj[rows].T).astype(bf),
            Wcat=Wcat.astype(bf), Asrc=Asrc.astype(bf), Adst=Adst.astype(bf),
            Wout=Wout_bf, aout2=aout2,
            WsT=WsT, ident=ident, selbc=selbc,
            idx1=gidx(pair1_idx[c * PC:(c + 1) * PC]),
            idx2=gidx(pair2_idx[c * PC:(c + 1) * PC]),
        ))
    return in_maps


_CACHE = {}


def _get_compiled(stage=99, iters=1):
    key = f"nc{stage}_{iters}"
    if key not in _CACHE:
        nc = _build_nc(stage, iters)
        nc.compile()
        _CACHE[key] = nc
    return _CACHE[key]


def kernel(**inputs):
    from concourse.bass_utils import run_bass_kernel_spmd

    nc = _get_compiled()
    in_maps = _make_in_maps(**inputs)
    res = run_bass_kernel_spmd(nc, in_maps, core_ids=list(range(NCORES)))
    out = np.concatenate(
        [np.asarray(res.results[c]["scores"], dtype=np.float32).reshape(PC)
         for c in range(NCORES)])
    return out



# revision 30
# speedup vs baseline: 1.1717x; 1.1717x over previous
"""GAT (2-layer graph attention + pair scoring) on 8 TRN2 NeuronCores.

Sharding: destination-node rows (4096/8=512 per core). Per layer, each core
computes Wh for its rows (bf16 matmuls), scales by q=exp(0.2*e_dst),
AllGathers the scaled [N, nhid(+aux)] matrix, then computes masked attention
for its 512 rows against all 4096 sources. Pair scoring shards the 65536
pairs over cores and gathers embeddings with chunked SWDGE dma_gather.

Key algebra: exp(leaky_relu(s)) with s = e_src_i + e_dst_j factors as
  p_i * q_j * max(a_i*b_j, 1),  a=exp(.8 e_src), b=exp(.8 e_dst),
  p=exp(.2 e_src), q=exp(.2 e_dst)
and p_i cancels between softmax numerator and denominator. So the N^2 stage
needs NO transcendentals: one dual-op tensor_scalar (mult+max, DVE 4x mode)
and one tensor_tensor (mask multiply) per tile. The softmax denominator
rides along as an extra q-column in the matmul's stationary operand.

Layer-1 AG payload: per-head blocks [Whq_h (64) | q_h] then 8 b columns.
Each head's attention matmul (lhsT = [Whq_h | q_h]) leaves numerators on PSUM
partitions 0..63 and the softmax denominator on partition 64. The mask
multiplies are split DVE/GPSIMD (heads 3,7 + half of 5 on GPSIMD); PSUM
evacuation rides the otherwise-idle Activation engine.

Final embeddings go out as ONE fused bf16 AllGather [N, 128] = [h | h@Ws^T];
pair rows come back via dma_gather in 512-index chunks (HW SWDGE ring limit)
alternating two SWDGE queues, ~7x faster than per-128-row indirect DMAs.
"""

import sys

if "/opt/trn_rl_repo" not in sys.path:
    sys.path.insert(0, "/opt/trn_rl_repo")

import numpy as np
import ml_dtypes

import concourse.bacc as bacc
import concourse.tile as tile
import concourse.mybir as mybir

BF16 = mybir.dt.bfloat16
F32 = mybir.dt.float32
I16 = mybir.dt.int16
AF = mybir.ActivationFunctionType
OP = mybir.AluOpType
AX = mybir.AxisListType

N, NFEAT, NHID, NHEADS = 4096, 512, 64, 8
P = 65536
NCORES = 8
R = N // NCORES          # rows (destination nodes) per core = 512
JT = N // 128            # source j-tiles = 32
PC = P // NCORES         # pairs per core = 8192
CH = PC // 128           # pair chunks = 64
HB = NHID + 1            # per-head AG1 block [Whq (64) | q] = 65
AG1C = NHEADS * HB + NHEADS   # 520 + 8 trailing b columns = 528
AG2C = NHID + 2          # [Whq2 (64) | q2 | b2] = 66

# heads whose mask-multiply runs on GPSIMD instead of DVE (load balance)
GP_HEADS = (3, 7)
GP_JT2 = 3               # in layer 2, every GP_JT2-th j-tile's mask-mul on gpsimd


GCHK = 1024              # indices per dma_gather call (HW limit: 2048 crashes)
GNC = PC // GCHK         # gather chunks per table = 8


def _build_nc(stage=99, iters=1):
    nc = bacc.Bacc("TRN2", target_bir_lowering=False, debug=False,
                   num_devices=NCORES, num_swdge_queues=2)

    def inp(name, shape, dt):
        return nc.dram_tensor(name, shape, dt, kind="ExternalInput").ap()

    xT = inp("xT", [NFEAT, R], BF16)           # x[rows].T  (feature-major)
    maskT = inp("maskT", [N, R], BF16)         # adj[rows].T (0/1)
    Wcat = inp("Wcat", [NFEAT, NHEADS * NHID], BF16)
    Asrc = inp("Asrc", [NHEADS * NHID, NHEADS], BF16)  # block-diag a_src
    Adst = inp("Adst", [NHEADS * NHID, NHEADS], BF16)  # block-diag a_dst
    Wout = inp("Wout", [NHEADS * NHID, NHID], BF16)
    aout2 = inp("aout2", [NHID, 2], BF16)      # col0 = a_out[:64], col1 = a_out[64:]
    WsT = inp("WsT", [NHID, NHID], F32)        # W_score.T
    ident = inp("ident", [128, 128], F32)
    selbc = inp("selbc", [NHEADS, R], F32)     # selbc[h, m] = (m//64 == h)
    idx1 = inp("idx1", [128, PC // 16], I16)   # dma_gather 16-partition wrap
    idx2 = inp("idx2", [128, PC // 16], I16)

    scores = nc.dram_tensor("scores", [128, CH], F32, kind="ExternalOutput").ap()

    rg = [list(range(NCORES))]

    with tile.TileContext(nc) as tc:
        with tc.tile_pool(name="sb", bufs=1) as sb, \
             tc.tile_pool(name="sbw", bufs=12) as sbw, \
             tc.tile_pool(name="ps", bufs=8, space="PSUM") as ps, \
             tc.tile_pool(name="dram", bufs=1, space="DRAM") as dram:

            for _it in range(iters):
                def pst(name):
                    return ps.tile([128, R], F32, tag="ps", name=name)

                # ---------- persistent loads ----------
                Wout_sb = sb.tile([128, 4, NHID], BF16, tag="Wout")
                nc.sync.dma_start(Wout_sb[:], Wout.rearrange("(k p) c -> p k c", p=128))
                aout2_sb = sb.tile([NHID, 2], BF16, tag="aout2")
                nc.sync.dma_start(aout2_sb[:], aout2[:])
                WsT_sb = sb.tile([NHID, NHID], F32, tag="WsT")
                nc.sync.dma_start(WsT_sb[:], WsT[:])
                ident_sb = sb.tile([128, 128], F32, tag="ident")
                nc.sync.dma_start(ident_sb[:], ident[:])
                selbc_sb = sb.tile([NHEADS, R], F32, tag="selbc")
                nc.sync.dma_start(selbc_sb[:], selbc[:])
                ones_sb = sb.tile([65, 128], F32, tag="ones")
                nc.vector.memset(ones_sb[:], 1.0)


                abc_sb = sb.tile([128, NHEADS, R], BF16, tag="abc")
                ag1_sb = [sb.tile([128, JT // 4, AG1C], BF16,
                                  tag=f"ag1sb{_q}", name=f"ag1sb{_q}")
                          for _q in range(4)]
                bf_sb = sb.tile([128, JT, NHEADS], F32, tag="bf")
                hcatT_sb = [sb.tile([128, R], BF16, tag=f"hcatT{_g}",
                                    name=f"hcatT{_g}") for _g in range(4)]
                num_sb = sb.tile([128, 4, R], BF16, tag="num")

                ag1_in = dram.tile([R, AG1C], BF16, tag="ag1in")
                ag1_out = dram.tile([N, AG1C], BF16, tag="ag1out", addr_space="Shared")

                # ---------- Phase A: local Wh / e / exps / AG1 payload ----------
                with tc.tile_pool(name="sbA", bufs=1) as sbA:
                    xT_sb = sbA.tile([128, 4, R], BF16, tag="xT")
                    nc.sync.dma_start(
                        xT_sb[:, 0:2, :],
                        xT[0:NFEAT // 2].rearrange("(k p) c -> p k c", p=128))
                    nc.sync.dma_start(
                        xT_sb[:, 2:4, :],
                        xT[NFEAT // 2:NFEAT].rearrange("(k p) c -> p k c", p=128))
                    Wcat_sb = sbA.tile([128, 4, NHEADS * NHID], BF16, tag="Wcat")
                    nc.sync.dma_start(
                        Wcat_sb[:, 0:2, :],
                        Wcat[0:NFEAT // 2].rearrange("(k p) c -> p k c", p=128))
                    nc.sync.dma_start(
                        Wcat_sb[:, 2:4, :],
                        Wcat[NFEAT // 2:NFEAT].rearrange("(k p) c -> p k c", p=128))
                    Asrc_sb = sbA.tile([128, 4, NHEADS], BF16, tag="Asrc")
                    nc.sync.dma_start(Asrc_sb[:],
                                      Asrc.rearrange("(k p) c -> p k c", p=128))
                    Adst_sb = sbA.tile([128, 4, NHEADS], BF16, tag="Adst")
                    nc.sync.dma_start(Adst_sb[:],
                                      Adst.rearrange("(k p) c -> p k c", p=128))

                    # bulk loads not needed until phase B, issued after
                    # phase A's inputs so they don't gate the first matmuls
                    maskT_sb = sb.tile([128, JT, R], BF16, tag="maskT")
                    nc.sync.dma_start(maskT_sb[:],
                                      maskT.rearrange("(j p) c -> p j c", p=128))
                    idx1_sb = sb.tile([128, PC // 16], I16, tag="idx1")
                    nc.sync.dma_start(idx1_sb[:], idx1[:])
                    idx2_sb = sb.tile([128, PC // 16], I16, tag="idx2")
                    nc.sync.dma_start(idx2_sb[:], idx2[:])

                    # Wh row-major [512_i, 512_hd] and WhT [512_hd, 512_i]
                    wh_sb = sbA.tile([128, 4, NHEADS * NHID], BF16, tag="wh")
                    whT_sb = sbA.tile([128, 4, R], BF16, tag="whT")
                    for m in range(4):
                        wh_ps = pst(f"whps{m}")
                        for k in range(4):
                            nc.tensor.matmul(wh_ps[:],
                                             xT_sb[:, k, m * 128:(m + 1) * 128],
                                             Wcat_sb[:, k, :],
                                             start=(k == 0), stop=(k == 3))
                        nc.scalar.copy(wh_sb[:, m, :], wh_ps[:])
                        whT_ps = pst(f"whTps{m}")
                        for k in range(4):
                            nc.tensor.matmul(whT_ps[:],
                                             Wcat_sb[:, k, m * 128:(m + 1) * 128],
                                             xT_sb[:, k, :],
                                             start=(k == 0), stop=(k == 3))
                        nc.scalar.copy(whT_sb[:, m, :], whT_ps[:])

                    # abc[h] = exp(0.8*e_src_h) broadcast to all partitions:
                    # stride-0 lhsT makes every output partition identical
                    for h in range(NHEADS):
                        ebc_ps = pst(f"ebc{h}")
                        for k in range(4):
                            nc.tensor.matmul(
                                ebc_ps[:],
                                Asrc_sb[:, k, h:h + 1].to_broadcast([128, 128]),
                                whT_sb[:, k, :],
                                start=(k == 0), stop=(k == 3))
                        nc.scalar.activation(abc_sb[:, h, :], ebc_ps[:], AF.Exp,
                                             scale=0.8)

                    # e_dstT [512_i, 8] -> qT (f32, scalar use) and bT (bf16)
                    qT_sb = sbA.tile([128, 4, NHEADS], F32, tag="qT")
                    bT_sb = sbA.tile([128, 4, NHEADS], BF16, tag="bT")
                    for m in range(4):
                        ed_ps = pst(f"ed{m}")
                        for k in range(4):
                            nc.tensor.matmul(ed_ps[:, 0:NHEADS],
                                             whT_sb[:, k, m * 128:(m + 1) * 128],
                                             Adst_sb[:, k, :],
                                             start=(k == 0), stop=(k == 3))
                        nc.scalar.activation(qT_sb[:, m, :], ed_ps[:, 0:NHEADS],
                                             AF.Exp, scale=0.2)
                        nc.scalar.activation(bT_sb[:, m, :], ed_ps[:, 0:NHEADS],
                                             AF.Exp, scale=0.8)

                    # AG1 payload: per-head [Whq_h | q_h] blocks + 8 b cols;
                    # per-m tiles so each chunk's DMA streams out while the
                    # next chunk is still being assembled
                    for m in range(4):
                        pay1m = sbA.tile([128, AG1C], BF16, tag=f"pay1{m}",
                                         name=f"pay1{m}")
                        for h in range(NHEADS):
                            nc.vector.tensor_scalar(
                                pay1m[:, h * HB:h * HB + NHID],
                                wh_sb[:, m, h * NHID:(h + 1) * NHID],
                                qT_sb[:, m, h:h + 1], None, OP.mult)
                            nc.vector.tensor_copy(
                                pay1m[:, h * HB + NHID:h * HB + NHID + 1],
                                qT_sb[:, m, h:h + 1])
                        nc.vector.tensor_copy(pay1m[:, NHEADS * HB:AG1C],
                                              bT_sb[:, m, :])
                        nc.sync.dma_start(ag1_in[m * 128:(m + 1) * 128, :],
                                          pay1m[:])

                nc.gpsimd.collective_compute(
                    "AllGather", OP.bypass, replica_groups=rg,
                    ins=[ag1_in.opt()], outs=[ag1_out.opt()])
                for _q in range(4):
                    nc.sync.dma_start(
                        ag1_sb[_q][:],
                        ag1_out[_q * (N // 4):(_q + 1) * (N // 4)].rearrange(
                            "(j p) c -> p j c", p=128))
                    nc.scalar.copy(
                        bf_sb[:, _q * (JT // 4):(_q + 1) * (JT // 4), :],
                        ag1_sb[_q][:, :, NHEADS * HB:AG1C])

                if stage == 1:
                    dmy = sb.tile([128, CH], F32, tag="dmy", name="dmy1")
                    nc.vector.memset(dmy[:], 0.0)
                    nc.sync.dma_start(scores[:], dmy[:])
                    return nc

                # ---------- Phase B: layer-1 attention ----------
                # lhsT = [Whq_h | q_h]: numerators on psum p0..63, denom on p64
                hp_ps = [ps.tile([65, R], F32, tag="ps", name=f"hp{h}")
                         for h in range(NHEADS)]
                for j in range(JT):
                    for h in range(NHEADS):
                        t = sbw.tile([128, R], BF16, tag="t")
                        nc.vector.tensor_scalar(
                            t[:], abc_sb[:, h, :], bf_sb[:, j, h:h + 1],
                            1.0, OP.mult, OP.max)
                        t2 = sbw.tile([128, R], BF16, tag="t2")
                        on_gp = h in GP_HEADS or (h == 5 and j % 2 == 0)
                        eng = nc.gpsimd if on_gp else nc.vector
                        eng.tensor_tensor(t2[:], t[:], maskT_sb[:, j, :], OP.mult)
                        nc.tensor.matmul(
                            hp_ps[h][:],
                            ag1_sb[j // (JT // 4)][:, j % (JT // 4),
                                                   h * HB:h * HB + NHID + 1],
                            t2[:], start=(j == 0), stop=(j == JT - 1))


                if stage == 2:
                    dmy = sb.tile([128, CH], F32, tag="dmy", name="dmy2")
                    nc.vector.memset(dmy[:], 0.0)
                    nc.sync.dma_start(scores[:], dmy[:])
                    return nc
                # normalize + elu -> hcatT [512_hd, 512_i] bf16
                with tc.tile_pool(name="sbE", bufs=1) as sbE:
                    den_st = sbE.tile([65, NHEADS, R], F32, tag="den_st")
                    for h in range(NHEADS):
                        nc.scalar.copy(den_st[64:65, h, :],
                                       hp_ps[h][64:65, :])
                        if h % 2 == 0:
                            nc.scalar.copy(num_sb[0:64, h // 2, :],
                                           hp_ps[h][0:64, :])
                        else:
                            nc.vector.tensor_copy(num_sb[64:128, h // 2, :],
                                                  hp_ps[h][0:64, :])
                    rin_sb = sbE.tile([NHEADS, R], F32, tag="rin")
                    nc.sync.dma_start(rin_sb[:], den_st[64:65, :, :])
                    rcp_sb = sbE.tile([NHEADS, R], F32, tag="rcp")
                    nc.vector.reciprocal(rcp_sb[:], rin_sb[:])

                    # per-chunk tiles: norm+elu stream into phase C's
                    # accumulating matmuls chunk-by-chunk
                    with tc.tile_pool(name="sbG", bufs=2) as sbG:
                        for g in range(4):
                            rbc_ps = pst(f"rbc{g}")
                            nc.tensor.matmul(rbc_ps[:],
                                             selbc_sb[:, g * 128:(g + 1) * 128],
                                             rcp_sb[:], start=True, stop=True)
                            xbg = sbG.tile([128, R], BF16, tag="xbg",
                                           name=f"xbg{g}")
                            nc.vector.tensor_tensor(xbg[:], num_sb[:, g, :],
                                                    rbc_ps[:], OP.mult)
                            # elu(x) = relu(x) + exp(min(x,0)) - 1
                            tmin = sbG.tile([128, R], BF16, tag="tming",
                                            name=f"tmin{g}")
                            nc.vector.tensor_scalar(tmin[:], xbg[:], 0.0, None,
                                                    OP.min)
                            texp = sbG.tile([128, R], BF16, tag="texpg",
                                            name=f"texp{g}")
                            nc.scalar.activation(texp[:], tmin[:], AF.Exp)
                            trel = sbG.tile([128, R], BF16, tag="trelg",
                                            name=f"trel{g}")
                            nc.vector.tensor_scalar(trel[:], xbg[:], 0.0, 1.0,
                                                    OP.max, OP.subtract)
                            nc.vector.tensor_tensor(hcatT_sb[g][:], texp[:],
                                                    trel[:], OP.add)

                # ---------- Phase C: layer-2 ingredients + AG2 ----------
                ag2_in = dram.tile([R, AG2C], BF16, tag="ag2in")
                ag2_out = dram.tile([N, AG2C], BF16, tag="ag2out", addr_space="Shared")
                wh2T_sb = sb.tile([NHID, R], BF16, tag="wh2Tsb")
                a2bc_sb = sb.tile([128, R], BF16, tag="a2bcsb")
                with tc.tile_pool(name="sbC", bufs=1) as sbC:
                    wh2T_ps = ps.tile([NHID, R], F32, tag="ps", name="wh2T")
                    for k in range(4):
                        nc.tensor.matmul(wh2T_ps[:], Wout_sb[:, k, :],
                                         hcatT_sb[k][:],
                                         start=(k == 0), stop=(k == 3))
                    nc.scalar.copy(wh2T_sb[:], wh2T_ps[:])

                    a2e_ps = pst("a2e")
                    nc.tensor.matmul(a2e_ps[:],
                                     aout2_sb[:, 0:1].to_broadcast([NHID, 128]),
                                     wh2T_sb[:], start=True, stop=True)
                    nc.scalar.activation(a2bc_sb[:], a2e_ps[:], AF.Exp, scale=0.8)

                    # AG2 payload block: [Whq2 (64) | q2 | b2]
                    pay2_sb = sbC.tile([128, 4, AG2C], BF16, tag="pay2")
                    for m in range(4):
                        wh2_ps = ps.tile([128, NHID], F32, tag="ps",
                                         name=f"wh2_{m}")
                        for k in range(4):
                            nc.tensor.matmul(wh2_ps[:],
                                             hcatT_sb[k][:, m * 128:(m + 1) * 128],
                                             Wout_sb[:, k, :],
                                             start=(k == 0), stop=(k == 3))
                        ed2_ps = ps.tile([128, 1], F32, tag="ps", name=f"ed2_{m}")
                        nc.tensor.matmul(ed2_ps[:],
                                         wh2T_sb[:, m * 128:(m + 1) * 128],
                                         aout2_sb[:, 1:2], start=True, stop=True)
                        q2f = sbC.tile([128, 4], F32, tag="q2f", bufs=4,
                                       name=f"q2f_{m}")
                        nc.scalar.activation(q2f[:, 0:1], ed2_ps[:], AF.Exp,
                                             scale=0.2)
                        nc.vector.tensor_copy(pay2_sb[:, m, NHID:NHID + 1],
                                              q2f[:, 0:1])
                        nc.scalar.activation(pay2_sb[:, m, NHID + 1:NHID + 2],
                                             ed2_ps[:], AF.Exp, scale=0.8)
                        nc.vector.tensor_scalar(pay2_sb[:, m, 0:NHID],
                                                wh2_ps[:], q2f[:, 0:1],
                                                None, OP.mult)
                    nc.sync.dma_start(
                        ag2_in[:].rearrange("(m p) c -> p m c", p=128), pay2_sb[:])

                nc.gpsimd.collective_compute(
                    "AllGather", OP.bypass, replica_groups=rg,
                    ins=[ag2_in.opt()], outs=[ag2_out.opt()])
                ag2_sb = sb.tile([128, JT, AG2C], BF16, tag="ag2sb")
                nc.sync.dma_start(ag2_sb[:],
                                  ag2_out[:].rearrange("(j p) c -> p j c", p=128))
                b2f_sb = sb.tile([128, JT, 1], F32, tag="b2f")
                nc.scalar.copy(b2f_sb[:], ag2_sb[:, :, NHID + 1:NHID + 2])

                if stage == 3:
                    dmy = sb.tile([128, CH], F32, tag="dmy", name="dmy3")
                    nc.vector.memset(dmy[:], 0.0)
                    nc.sync.dma_start(scores[:], dmy[:])
                    return nc

                # ---------- Phase D: layer-2 attention ----------
                hp2_ps = ps.tile([65, R], F32, tag="ps", name="hp2")
                for j in range(JT):
                    t = sbw.tile([128, R], BF16, tag="t")
                    nc.vector.tensor_scalar(t[:], a2bc_sb[:], b2f_sb[:, j, 0:1],
                                            1.0, OP.mult, OP.max)
                    t2 = sbw.tile([128, R], BF16, tag="t2")
                    eng = nc.gpsimd if (j % GP_JT2 == GP_JT2 - 1) else nc.vector
                    eng.tensor_tensor(t2[:], t[:], maskT_sb[:, j, :], OP.mult)
                    nc.tensor.matmul(hp2_ps[:], ag2_sb[:, j, 0:NHID + 1], t2[:],
                                     start=(j == 0), stop=(j == JT - 1))


                if stage == 4:
                    dmy = sb.tile([128, CH], F32, tag="dmy", name="dmy4")
                    nc.vector.memset(dmy[:], 0.0)
                    nc.sync.dma_start(scores[:], dmy[:])
                    return nc
                hfT_sb = sb.tile([NHID, R], F32, tag="hfT")
                with tc.tile_pool(name="sbD", bufs=1) as sbD:
                    rcp2_sb = sbD.tile([65, R], F32, tag="rcp2")
                    nc.vector.reciprocal(rcp2_sb[64:65, :], hp2_ps[64:65, :])
                    rbc2_ps = ps.tile([NHID, R], F32, tag="ps", name="rbc2")
                    nc.tensor.matmul(rbc2_ps[:], ones_sb[64:65, 0:NHID],
                                     rcp2_sb[64:65, :], start=True, stop=True)
                    num2_sb = sbD.tile([NHID, R], F32, tag="num2")
                    nc.scalar.copy(num2_sb[:], hp2_ps[0:NHID, :])
                    xn2_sb = sbD.tile([NHID, R], F32, tag="xn2")
                    nc.vector.tensor_tensor(xn2_sb[:], num2_sb[:],
                                            rbc2_ps[:], OP.mult)
                    # elu in f32
                    tmin2 = sbD.tile([NHID, R], F32, tag="tmin2")
                    nc.vector.tensor_scalar(tmin2[:], xn2_sb[:], 0.0, None, OP.min)
                    texp2 = sbD.tile([NHID, R], F32, tag="texp2")
                    nc.scalar.activation(texp2[:], tmin2[:], AF.Exp)
                    trel2 = sbD.tile([NHID, R], F32, tag="trel2")
                    nc.vector.tensor_scalar(trel2[:], xn2_sb[:], 0.0, 1.0, OP.max,
                                            OP.subtract)
                    nc.vector.tensor_tensor(hfT_sb[:], texp2[:], trel2[:], OP.add)

                # ---------- Phase E: H2 = h @ Ws^T, transpose h, AG3 ----------
                # single fused collective: cols [0:64] = h rows, [64:128] = h@Ws^T
                ag3_in = dram.tile([R, 2 * NHID], BF16, tag="ag3in")
                ag3_out = dram.tile([N, 2 * NHID], BF16, tag="ag3out",
                                    addr_space="Shared")
                with tc.tile_pool(name="sbF", bufs=1) as sbF:
                    ag3_sb = sbF.tile([128, 4, 2 * NHID], BF16, tag="ag3")
                    for m in range(4):
                        h2_ps = ps.tile([128, NHID], F32, tag="ps", name=f"h2_{m}")
                        nc.tensor.matmul(h2_ps[:], hfT_sb[:, m * 128:(m + 1) * 128],
                                         WsT_sb[:], start=True, stop=True)
                        nc.scalar.copy(ag3_sb[:, m, NHID:2 * NHID], h2_ps[:])
                        hf_ps = ps.tile([128, NHID], F32, tag="ps", name=f"hf_{m}")
                        nc.tensor.transpose(hf_ps[:],
                                            hfT_sb[:, m * 128:(m + 1) * 128],
                                            ident_sb[0:NHID, 0:NHID])
                        nc.scalar.copy(ag3_sb[:, m, 0:NHID], hf_ps[:])

                    nc.sync.dma_start(
                        ag3_in[:].rearrange("(m p) c -> p m c", p=128), ag3_sb[:])
                    nc.gpsimd.collective_compute(
                        "AllGather", OP.bypass, replica_groups=rg,
                        ins=[ag3_in.opt()], outs=[ag3_out.opt()])


                    if stage == 5:
                        dmy = sbF.tile([128, CH], F32, tag="dmy", name="dmy5")
                        nc.vector.memset(dmy[:], 0.0)
                        nc.sync.dma_start(scores[:], dmy[:])
                        return nc
                    # ---------- Phase F: pair gather + bilinear score ----------
                    # dma_gather: 512-idx chunks (SWDGE ring limit), the two
                    # tables on separate SWDGE queues for parallel desc-gen
                    # per-quarter tiles: Tile tracks them independently, so
                    # each quarter's score math overlaps later quarters'
                    # gather descriptor-gen on the Pool engine
                    g1_sb = [sbF.tile([128, CH // 4, 2 * NHID], BF16,
                                      tag=f"g1_{_q}", name=f"g1_{_q}")
                             for _q in range(4)]
                    g2_sb = [sbF.tile([128, CH // 4, 2 * NHID], BF16,
                                      tag=f"g2_{_q}", name=f"g2_{_q}")
                             for _q in range(4)]
                    for c in range(GNC):
                        q, cq = c // (GNC // 4), c % (GNC // 4)
                        cc = slice(cq * (GCHK // 128), (cq + 1) * (GCHK // 128))
                        ci = slice(c * (GCHK // 16), (c + 1) * (GCHK // 16))
                        nc.gpsimd.dma_gather(
                            g1_sb[q][:, cc, :], ag3_out[:], idx1_sb[:, ci],
                            GCHK, GCHK, 2 * NHID, queue_num=0)
                        nc.gpsimd.dma_gather(
                            g2_sb[q][:, cc, :], ag3_out[:], idx2_sb[:, ci],
                            GCHK, GCHK, 2 * NHID, queue_num=1)

                    if stage == 6:
                        dmy = sbF.tile([128, CH], F32, tag="dmy", name="dmy6")
                        nc.vector.memset(dmy[:], 0.0)
                        nc.sync.dma_start(scores[:], dmy[:])
                        return nc
                    sc_sb = sbF.tile([128, CH], F32, tag="sc")
                    for q in range(4):
                        prod_sb = sbF.tile([128, CH // 4, NHID], BF16,
                                           tag=f"prod{q}", name=f"prod{q}")
                        nc.vector.tensor_tensor(prod_sb[:],
                                                g1_sb[q][:, :, 0:NHID],
                                                g2_sb[q][:, :, NHID:2 * NHID],
                                                OP.mult)
                        nc.vector.tensor_reduce(
                            sc_sb[:, q * (CH // 4):(q + 1) * (CH // 4)],
                            prod_sb[:], AX.X, OP.add)
                    nc.sync.dma_start(scores[:], sc_sb[:])

    return nc


def _make_in_maps(x, adj, W_heads, a_heads, W_out, a_out, W_score,
                  pair1_idx, pair2_idx):
    bf = ml_dtypes.bfloat16
    x = np.asarray(x, dtype=np.float32)
    adj = np.asarray(adj, dtype=np.float32)
    W_heads = np.asarray(W_heads, dtype=np.float32)
    a_heads = np.asarray(a_heads, dtype=np.float32)
    W_out = np.asarray(W_out, dtype=np.float32)
    a_out = np.asarray(a_out, dtype=np.float32)
    W_score = np.asarray(W_score, dtype=np.float32)
    pair1_idx = np.asarray(pair1_idx, dtype=np.int32)
    pair2_idx = np.asarray(pair2_idx, dtype=np.int32)

    Wcat = np.concatenate([W_heads[h] for h in range(NHEADS)], axis=1)
    Wcat = np.ascontiguousarray(Wcat, dtype=np.float32)
    Asrc = np.zeros((NHEADS * NHID, NHEADS), dtype=np.float32)
    Adst = np.zeros((NHEADS * NHID, NHEADS), dtype=np.float32)
    for h in range(NHEADS):
        Asrc[h * NHID:(h + 1) * NHID, h] = a_heads[h, :NHID]
        Adst[h * NHID:(h + 1) * NHID, h] = a_heads[h, NHID:]
    Wout_bf = W_out.astype(bf)
    aout2 = np.stack([a_out[:NHID], a_out[NHID:]], axis=1).astype(bf)
    WsT = np.ascontiguousarray(W_score.T, dtype=np.float32)
    ident = np.eye(128, dtype=np.float32)
    selbc = np.zeros((NHEADS, R), dtype=np.float32)
    for h in range(NHEADS):
        selbc[h, h * NHID:(h + 1) * NHID] = 1.0

    # dma_gather slot i lands at dst[i % 128, (i // 512) * 4 + (i % 512) // 128]
    # permute so dst[p, ch] = pair p * CH + ch (same layout the unpack expects),
    # then wrap in the 16-partition index layout replicated over 8 groups
    i_arr = np.arange(PC)
    gperm = (i_arr % 128) * CH + (i_arr // GCHK) * (GCHK // 128) \
        + (i_arr % GCHK) // 128

    def gidx(ids):
        g = ids[gperm].astype(np.int16)
        return np.ascontiguousarray(
            np.tile(g.reshape(PC // 16, 16).T, (8, 1)))

    in_maps = []
    for c in range(NCORES):
        rows = slice(c * R, (c + 1) * R)
        in_maps.append(dict(
            xT=np.ascontiguousarray(x[rows].T).astype(bf),
            maskT=np.ascontiguousarray(adj[rows].T).astype(bf),
            Wcat=Wcat.astype(bf), Asrc=Asrc.astype(bf), Adst=Adst.astype(bf),
            Wout=Wout_bf, aout2=aout2,
            WsT=WsT, ident=ident, selbc=selbc,
            idx1=gidx(pair1_idx[c * PC:(c + 1) * PC]),
            idx2=gidx(pair2_idx[c * PC:(c + 1) * PC]),
        ))
    return in_maps


_CACHE = {}


def _get_compiled(stage=99, iters=1):
    key = f"nc{stage}_{iters}"
    if key not in _CACHE:
        nc = _build_nc(stage, iters)
        nc.compile()
        _CACHE[key] = nc
    return _CACHE[key]


def kernel(**inputs):
    from concourse.bass_utils import run_bass_kernel_spmd

    nc = _get_compiled()
    in_maps = _make_in_maps(**inputs)
    res = run_bass_kernel_spmd(nc, in_maps, core_ids=list(range(NCORES)))
    out = np.concatenate(
        [np.asarray(res.results[c]["scores"], dtype=np.float32).reshape(PC)
         for c in range(NCORES)])
    return out

